# revision 24
# baseline (speedup 1.0000x reference)
"""MoE routing layer kernel for 8 Trainium2 NeuronCores.

Math (matching the reference exactly):
  logits  = x @ gate_w.T + gate_b + expert_biases            [BS, 8]
  probs   = sigmoid(logits); top2 by value (ties -> lower index)
  proj_j[t, e] = x[t] . expert_w[e, j, :] + expert_b[e, j]   (j = 0, 1)
  val_j   = proj_j[t, idx_j]
  out[t, :] = (val_0 * p_0 + val_1 * p_1) / (p_0 + p_1)      broadcast over 1024
Only rows 0..K-1 of each expert's weight matrix are ever used (the reference
gathers with the top-k slot as the feature index), so the device computes a
single fused 24-column matmul: [gate(8) | expert row0 (8) | expert row1 (8)].

Implementation notes:
- PE instructions cost ~400ns each regardless of size, so the design minimizes
  PE instruction count: x is split on the host into bf16 hi/lo halves
  (x == hi + lo to ~2^-18 relative), transposed tiles are loaded via the DMA
  transpose engine (2-byte dtypes only), and the fused weight matrix is split
  the same way, stacked [w_hi | w_lo] so one matmul produces both partial
  products. All four split products accumulate in PSUM in fp32; max logit
  error vs f64 is ~1e-5, ~9x below the smallest top-2 decision margin of the
  gating distribution, so top-k indices match the f32 reference exactly.
- Per-token ops run batched across the 8 token tiles with 3D access patterns.

Sharding: data-parallel over the 8192 tokens, 1024 tokens per core.
"""

import numpy as np
import ml_dtypes

import concourse.bass as bass
import concourse.bacc as bacc
import concourse.mybir as mybir
from concourse import bass_utils
from concourse.tile import TileContext
from concourse.tile_rust import add_dep_helper

# Problem shape (hardcoded per contract).
B, S, D, E, K = 4, 2048, 1024, 8, 2
N_CORES = 8
TOK = B * S                   # 8192 tokens total
TPC = TOK // N_CORES          # 1024 tokens per core
P = 128                       # partitions
NT = TPC // P                 # 8 token tiles per core
DC = D // P                   # 8 contraction chunks
W24 = 3 * E                   # 24 fused output columns
W32 = 32                      # partition-aligned block (PSUM reads need 32-aligned starts)
W64 = 2 * W32                 # hi+lo stacked, padded

F32 = mybir.dt.float32
BF16 = mybir.dt.bfloat16
I32 = mybir.dt.int32
U32 = mybir.dt.uint32
AX = mybir.AxisListType
ALU = mybir.AluOpType
ACTF = mybir.ActivationFunctionType


def build_kernel_body(nc, tc, ins, outs):
    xh, xl, wt, cst = ins["xh"], ins["xl"], ins["wt"], ins["cst"]
    out, idx = outs["out"], outs["idx"]

    from contextlib import ExitStack
    with ExitStack() as ctx:
        const = ctx.enter_context(tc.tile_pool(name="const", bufs=1))
        xtp = ctx.enter_context(tc.tile_pool(name="xt", bufs=1))
        pjps = ctx.enter_context(tc.tile_pool(name="pjp", bufs=1, space="PSUM"))
        btps = ctx.enter_context(tc.tile_pool(name="btp", bufs=6, space="PSUM"))
        spool = ctx.enter_context(tc.tile_pool(name="small", bufs=1))
        opool = ctx.enter_context(tc.tile_pool(name="obuf", bufs=NT))

        # small replicated tensors ride SWDGE so they don't block a
        # transpose queue: cst = [brep (24) | ident (128)] fp32
        cst_s = const.tile([P, W24 + P], F32, tag="cst")
        wt_s = const.tile([P, DC * W64], BF16, tag="wt")
        nc.gpsimd.dma_start(cst_s, cst)
        nc.gpsimd.dma_start(wt_s, wt)
        brep_s = cst_s[:, 0:W24]
        ident_s = cst_s[:, W24:W24 + P]

        # preload the Sigmoid activation table off the critical path (the
        # first use of a new func costs a ~1.3us ACT_TABLE_LOAD)
        sigwarm = const.tile([P, 1], F32, tag="sigwarm")
        nc.scalar.activation(sigwarm, cst_s[:, 0:1], ACTF.Sigmoid)

        # x^T tiles arrive pre-transposed from the host as plain contiguous
        # DMAs. Loaded per (chunk, token-half) block so half 0's matmul chain
        # completes while half 1 is still streaming in (per-queue DMA rate is
        # the bottleneck, ~46GB/s); separate tiles keep the deps block-level.
        NH = TPC // 512        # psum-bank-sized token halves
        xth = [[xtp.tile([P, 512], BF16, name=f"xth{c}_{h}", tag=f"xth{c}_{h}")
                for h in range(NH)] for c in range(DC)]
        xtl = [[xtp.tile([P, 512], BF16, name=f"xtl{c}_{h}", tag=f"xtl{c}_{h}")
                for h in range(NH)] for c in range(DC)]
        for h in range(NH):
            tok = slice(h * 512, (h + 1) * 512)
            for c in range(DC):
                nc.sync.dma_start(xth[c][h], xh[c][:, tok])
                nc.scalar.dma_start(xtl[c][h], xl[c][:, tok])

        # whole-kernel tiles for the per-token stage
        pjt = spool.tile([W24, TPC], F32, tag="pjt")
        plo = spool.tile([W24, TPC], F32, tag="plo")
        proj = spool.tile([P, NT * W24], F32, tag="proj")
        mx = spool.tile([P, NT * E], F32, tag="mx")
        mi = spool.tile([P, NT * E], U32, tag="mi")
        p12 = spool.tile([P, NT * 2], F32, tag="p12")
        msk = spool.tile([P, 2 * NT * E], F32, tag="msk")
        vv = spool.tile([P, 2 * NT * E], F32, tag="vv")
        val = spool.tile([P, 2 * NT], F32, tag="val")
        wv = spool.tile([P, 2 * NT], F32, tag="wv")
        num = spool.tile([P, NT], F32, tag="num")
        den = spool.tile([P, NT], F32, tag="den")
        rden = spool.tile([P, NT], F32, tag="rden")
        wgt = spool.tile([P, NT], F32, tag="wgt")
        ii = spool.tile([P, NT * 2], I32, tag="ii")

        proj3 = proj.rearrange("p (t w) -> p t w", t=NT)
        mx3 = mx.rearrange("p (t w) -> p t w", t=NT)
        mi3 = mi.rearrange("p (t w) -> p t w", t=NT)
        p123 = p12.rearrange("p (t w) -> p t w", t=NT)
        msk3 = msk.rearrange("p (j t w) -> p j t w", j=2, t=NT)
        vv3 = vv.rearrange("p (j t w) -> p j t w", j=2, t=NT)
        val3 = val.rearrange("p (j t) -> p j t", j=2)
        wv3 = wv.rearrange("p (j t) -> p j t", j=2)
        ii3 = ii.rearrange("p (t w) -> p t w", t=NT)
        idx3 = idx.rearrange("(t p) w -> p t w", p=P)

        TH = NT // NH          # token tiles per half
        # token-half-pipelined: half 0 streams out while half 1 computes
        for h in range(NH):
            tok = slice(h * 512, (h + 1) * 512)
            ts = slice(h * TH, (h + 1) * TH)

            # projT[64, tok] += [w_hi |pad| w_lo |pad].T @ xT_{hi,lo};
            # rows 0:24 collect w_hi.(x_hi+x_lo), rows 32:56 w_lo.(...)
            pj2 = pjps.tile([W64, 512], F32, name=f"pj{h}", tag=f"pj{h}")
            for c in range(DC):
                lhsT = wt_s[:, c * W64:(c + 1) * W64]
                nc.tensor.matmul(pj2, lhsT, xth[c][h],
                                 start=(c == 0), stop=False,
                                 skip_group_check=True)
                nc.tensor.matmul(pj2, lhsT, xtl[c][h],
                                 start=False, stop=(c == DC - 1),
                                 skip_group_check=True)

            # combine hi+lo rows (DVE may read only one PSUM operand per op);
            # the explicit dep keeps this half's combine from being scheduled
            # ahead of the previous half's broadcasts on the in-order DVE
            # queue (head-of-line blocking while waiting on this half's psum)
            cmb = nc.vector.tensor_copy(plo[:, tok], pj2[W32:W32 + W24, :])
            if h > 0:
                add_dep_helper(cmb.ins, last_bcast.ins,
                               reason="DVE order: prev-half stores first")
            nc.vector.tensor_add(pjt[:, tok], pj2[0:W24, :], plo[:, tok])

            # back to token-major, fusing the bias add into the PSUM move
            for t in range(h * TH, (h + 1) * TH):
                bt = btps.tile([P, W24], F32, tag="bt")
                nc.tensor.transpose(bt, pjt[:, t * P:(t + 1) * P],
                                    ident_s[0:W24, 0:W24])
                nc.vector.tensor_add(proj[:, t * W24:(t + 1) * W24], bt,
                                     brep_s)

            # top-2 over the 8 gate logits (monotone in probs); per tile
            # since MAX8/FIND_INDEX8 reduce the whole free dim
            for t in range(h * TH, (h + 1) * TH):
                g = proj[:, t * W24:t * W24 + E]
                nc.vector.max(out=mx[:, t * E:(t + 1) * E], in_=g)
                nc.vector.max_index(out=mi[:, t * E:(t + 1) * E],
                                    in_max=mx[:, t * E:(t + 1) * E],
                                    in_values=g)

            # batched across this half's tiles
            nc.scalar.activation(p123[:, ts], mx3[:, ts, 0:2], ACTF.Sigmoid)
            nc.vector.tensor_tensor(
                msk3[:, 0, ts], proj3[:, ts, 0:E],
                mx3[:, ts, 0:1].to_broadcast([P, TH, E]), op=ALU.is_equal)
            nc.vector.tensor_tensor(
                msk3[:, 1, ts], proj3[:, ts, 0:E],
                mx3[:, ts, 1:2].to_broadcast([P, TH, E]), op=ALU.is_equal)
            nc.vector.tensor_mul(vv3[:, 0, ts], proj3[:, ts, E:2 * E],
                                 msk3[:, 0, ts])
            nc.vector.tensor_mul(vv3[:, 1, ts], proj3[:, ts, 2 * E:3 * E],
                                 msk3[:, 1, ts])
            nc.vector.reduce_sum(val3[:, :, ts], vv3[:, :, ts], axis=AX.X)

            # weighted = (val0*p0 + val1*p1) / (p0 + p1)
            nc.vector.tensor_mul(wv3[:, 0, ts], val3[:, 0, ts],
                                 p123[:, ts, 0])
            nc.vector.tensor_mul(wv3[:, 1, ts], val3[:, 1, ts],
                                 p123[:, ts, 1])
            nc.vector.tensor_add(num[:, ts], wv3[:, 0, ts], wv3[:, 1, ts])
            nc.vector.tensor_add(den[:, ts], p123[:, ts, 0], p123[:, ts, 1])
            nc.vector.reciprocal(rden[:, ts], den[:, ts])
            nc.vector.tensor_mul(wgt[:, ts], num[:, ts], rden[:, ts])

            # indices out: u32 -> i32 cast, one DMA per half
            nc.vector.tensor_copy(ii3[:, ts], mi3[:, ts, 0:2])
            nc.gpsimd.dma_start(idx3[:, ts], ii3[:, ts])

            # broadcast across the 1024 output features and store. DVE only:
            # ACT Copies would thrash the activation table, and GpSimd copies
            # run ~4us and stall DVE via port sharing. Stores ride the HWDGE
            # engines, whose queues are free again by now.
            for t in range(h * TH, (h + 1) * TH):
                obuf = opool.tile([P, D], F32, tag="obuf")
                src = wgt[:, t:t + 1].to_broadcast([P, D])
                last_bcast = nc.vector.tensor_copy(obuf, src)
                r0, rm, r1 = t * P, t * P + P // 2, (t + 1) * P
                nc.gpsimd.dma_start(out[r0:rm, :], obuf[0:P // 2, :])
                nc.gpsimd.dma_start(out[rm:r1, :], obuf[P // 2:P, :])


def _prep_shared(gate_w, gate_b, expert_biases, expert_w, expert_b):
    """Host-side packing of the replicated small tensors."""
    w24 = np.concatenate(
        [gate_w, expert_w[:, 0, :], expert_w[:, 1, :]], axis=0)  # [24, 1024]
    w_hi = w24.astype(ml_dtypes.bfloat16)
    w_lo = (w24 - w_hi.astype(np.float32)).astype(ml_dtypes.bfloat16)
    pad = np.zeros((W32 - W24, D), ml_dtypes.bfloat16)
    w64 = np.concatenate([w_hi, pad, w_lo, pad], axis=0)         # [64, 1024]
    # wt[p, c*64 + j] = w64[j, c*128 + p]
    wt = np.ascontiguousarray(
        w64.T.reshape(DC, P, W64).transpose(1, 0, 2).reshape(P, DC * W64))
    b24 = np.concatenate(
        [gate_b + expert_biases, expert_b[:, 0], expert_b[:, 1]])  # [24]
    brep = np.broadcast_to(b24.astype(np.float32), (P, W24))
    ident = np.eye(P, dtype=np.float32)
    cst = np.ascontiguousarray(np.concatenate([brep, ident], axis=1))
    return wt, cst


def _build_module():
    nc = bacc.Bacc("TRN2", target_bir_lowering=False, debug=False,
                   num_devices=1)
    ins = {
        "xh": nc.dram_tensor("xh", [DC, P, TPC], BF16,
                             kind="ExternalInput").ap(),
        "xl": nc.dram_tensor("xl", [DC, P, TPC], BF16,
                             kind="ExternalInput").ap(),
        "wt": nc.dram_tensor("wt", [P, DC * W64], BF16,
                             kind="ExternalInput").ap(),
        "cst": nc.dram_tensor("cst", [P, W24 + P], F32,
                              kind="ExternalInput").ap(),
    }
    outs = {
        "out": nc.dram_tensor("out", [TPC, D], F32, kind="ExternalOutput").ap(),
        "idx": nc.dram_tensor("idx", [TPC, 2], I32, kind="ExternalOutput").ap(),
    }
    with TileContext(nc) as tc:
        build_kernel_body(nc, tc, ins, outs)
    nc.compile()
    return nc


_NC_CACHE = None


def _get_module():
    global _NC_CACHE
    if _NC_CACHE is None:
        _NC_CACHE = _build_module()
    return _NC_CACHE


def _run(inputs, trace=False, trace_kwargs=None):
    x = np.asarray(inputs["x"], np.float32)
    wt, cst = _prep_shared(
        np.asarray(inputs["gate_w"], np.float32),
        np.asarray(inputs["gate_b"], np.float32),
        np.asarray(inputs["expert_biases"], np.float32),
        np.asarray(inputs["expert_w"], np.float32),
        np.asarray(inputs["expert_b"], np.float32),
    )
    xf = x.reshape(TOK, D)
    x_hi = xf.astype(ml_dtypes.bfloat16)
    x_lo = (xf - x_hi.astype(np.float32)).astype(ml_dtypes.bfloat16)
    # pre-transposed chunk-major layout: xh[core][c][p][t] = x^T tiles
    x_hi = np.ascontiguousarray(
        x_hi.reshape(N_CORES, TPC, DC, P).transpose(0, 2, 3, 1))
    x_lo = np.ascontiguousarray(
        x_lo.reshape(N_CORES, TPC, DC, P).transpose(0, 2, 3, 1))
    shared = {"wt": wt, "cst": cst}
    in_maps = [
        {"xh": x_hi[c], "xl": x_lo[c], **shared}
        for c in range(N_CORES)
    ]
    nc = _get_module()
    kw = {}
    if trace:
        kw["trace"] = True
        kw["trace_cores"] = list(range(N_CORES))
        if trace_kwargs:
            kw["trace_kwargs"] = trace_kwargs
    res = bass_utils.run_bass_kernel_spmd(
        nc, in_maps, core_ids=list(range(N_CORES)), **kw)
    out = np.concatenate([res.results[c]["out"] for c in range(N_CORES)],
                         axis=0).reshape(B, S, D)
    idx = np.concatenate([res.results[c]["idx"] for c in range(N_CORES)],
                         axis=0).reshape(B, S, K)
    return (out.astype(np.float32), idx.astype(np.int32)), res


def kernel(**inputs):
    (out, idx), _ = _run(inputs)
    return out, idx


# revision 25
# speedup vs baseline: 1.1192x; 1.1192x over previous
"""MoE routing layer kernel for 8 Trainium2 NeuronCores.

Math (matching the reference exactly):
  logits  = x @ gate_w.T + gate_b + expert_biases            [BS, 8]
  probs   = sigmoid(logits); top2 by value (ties -> lower index)
  proj_j[t, e] = x[t] . expert_w[e, j, :] + expert_b[e, j]   (j = 0, 1)
  val_j   = proj_j[t, idx_j]
  out[t, :] = (val_0 * p_0 + val_1 * p_1) / (p_0 + p_1)      broadcast over 1024
Only rows 0..K-1 of each expert's weight matrix are ever used (the reference
gathers with the top-k slot as the feature index), so the device computes a
single fused 24-column matmul: [gate(8) | expert row0 (8) | expert row1 (8)].

Implementation notes:
- PE instructions cost ~400ns each regardless of size, so the design minimizes
  PE instruction count: x is split on the host into bf16 hi/lo halves
  (x == hi + lo to ~2^-18 relative), transposed tiles are loaded via the DMA
  transpose engine (2-byte dtypes only), and the fused weight matrix is split
  the same way, stacked [w_hi | w_lo] so one matmul produces both partial
  products. All four split products accumulate in PSUM in fp32; max logit
  error vs f64 is ~1e-5, ~9x below the smallest top-2 decision margin of the
  gating distribution, so top-k indices match the f32 reference exactly.
- Per-token ops run batched across the 8 token tiles with 3D access patterns.

Sharding: data-parallel over the 8192 tokens, 1024 tokens per core.
"""

import numpy as np
import ml_dtypes

import concourse.bass as bass
import concourse.bacc as bacc
import concourse.mybir as mybir
from concourse import bass_utils
from concourse.tile import TileContext
from concourse.tile_rust import add_dep_helper

# Problem shape (hardcoded per contract).
B, S, D, E, K = 4, 2048, 1024, 8, 2
N_CORES = 8
TOK = B * S                   # 8192 tokens total
TPC = TOK // N_CORES          # 1024 tokens per core
P = 128                       # partitions
NT = TPC // P                 # 8 token tiles per core
DC = D // P                   # 8 contraction chunks
W24 = 3 * E                   # 24 fused output columns
W32 = 32                      # partition-aligned block (PSUM reads need 32-aligned starts)
W64 = 2 * W32                 # hi+lo stacked, padded

F32 = mybir.dt.float32
BF16 = mybir.dt.bfloat16
I32 = mybir.dt.int32
U32 = mybir.dt.uint32
AX = mybir.AxisListType
ALU = mybir.AluOpType
ACTF = mybir.ActivationFunctionType


def build_kernel_body(nc, tc, ins, outs):
    xh, xl, wt, cst = ins["xh"], ins["xl"], ins["wt"], ins["cst"]
    out, idx = outs["out"], outs["idx"]

    from contextlib import ExitStack
    with ExitStack() as ctx:
        const = ctx.enter_context(tc.tile_pool(name="const", bufs=1))
        xtp = ctx.enter_context(tc.tile_pool(name="xt", bufs=1))
        pjps = ctx.enter_context(tc.tile_pool(name="pjp", bufs=1, space="PSUM"))
        btps = ctx.enter_context(tc.tile_pool(name="btp", bufs=6, space="PSUM"))
        spool = ctx.enter_context(tc.tile_pool(name="small", bufs=1))
        opool = ctx.enter_context(tc.tile_pool(name="obuf", bufs=NT))

        # small replicated tensors ride SWDGE so they don't block a
        # transpose queue: cst = [brep (24) | ident (128)] fp32
        cst_s = const.tile([P, W24 + P], F32, tag="cst")
        wt_s = const.tile([P, DC * W64], BF16, tag="wt")
        nc.gpsimd.dma_start(cst_s, cst)
        nc.gpsimd.dma_start(wt_s, wt)
        brep_s = cst_s[:, 0:W24]
        ident_s = cst_s[:, W24:W24 + P]

        # preload the Sigmoid activation table off the critical path (the
        # first use of a new func costs a ~1.3us ACT_TABLE_LOAD)
        sigwarm = const.tile([P, 1], F32, tag="sigwarm")
        nc.scalar.activation(sigwarm, cst_s[:, 0:1], ACTF.Sigmoid)

        # x^T tiles arrive pre-transposed from the host as plain contiguous
        # DMAs: one full-token block per (chunk, hi/lo) keeps queue handoffs
        # minimal so the 8 HW queues stream near their aggregate rate.
        NH = TPC // 512        # psum-bank-sized token halves
        xth = [xtp.tile([P, TPC], BF16, name=f"xth{c}", tag=f"xth{c}")
               for c in range(DC)]
        xtl = [xtp.tile([P, TPC], BF16, name=f"xtl{c}", tag=f"xtl{c}")
               for c in range(DC)]
        for c in range(DC):
            nc.sync.dma_start(xth[c], xh[c])
            nc.scalar.dma_start(xtl[c], xl[c])

        # whole-kernel tiles for the per-token stage
        pjt = spool.tile([W24, TPC], F32, tag="pjt")
        plo = spool.tile([W24, TPC], F32, tag="plo")
        proj = spool.tile([P, NT * W24], F32, tag="proj")
        mx = spool.tile([P, NT * E], F32, tag="mx")
        mi = spool.tile([P, NT * E], U32, tag="mi")
        p12 = spool.tile([P, NT * 2], F32, tag="p12")
        msk = spool.tile([P, 2 * NT * E], F32, tag="msk")
        vv = spool.tile([P, 2 * NT * E], F32, tag="vv")
        val = spool.tile([P, 2 * NT], F32, tag="val")
        wv = spool.tile([P, 2 * NT], F32, tag="wv")
        num = spool.tile([P, NT], F32, tag="num")
        den = spool.tile([P, NT], F32, tag="den")
        rden = spool.tile([P, NT], F32, tag="rden")
        wgt = spool.tile([P, NT], F32, tag="wgt")
        ii = spool.tile([P, NT * 2], I32, tag="ii")

        proj3 = proj.rearrange("p (t w) -> p t w", t=NT)
        mx3 = mx.rearrange("p (t w) -> p t w", t=NT)
        mi3 = mi.rearrange("p (t w) -> p t w", t=NT)
        p123 = p12.rearrange("p (t w) -> p t w", t=NT)
        msk3 = msk.rearrange("p (j t w) -> p j t w", j=2, t=NT)
        vv3 = vv.rearrange("p (j t w) -> p j t w", j=2, t=NT)
        val3 = val.rearrange("p (j t) -> p j t", j=2)
        wv3 = wv.rearrange("p (j t) -> p j t", j=2)
        ii3 = ii.rearrange("p (t w) -> p t w", t=NT)
        idx3 = idx.rearrange("(t p) w -> p t w", p=P)

        TH = NT // NH          # token tiles per half
        # token-half-pipelined: half 0 streams out while half 1 computes
        for h in range(NH):
            tok = slice(h * 512, (h + 1) * 512)
            ts = slice(h * TH, (h + 1) * TH)

            # projT[64, tok] += [w_hi |pad| w_lo |pad].T @ xT_{hi,lo};
            # rows 0:24 collect w_hi.(x_hi+x_lo), rows 32:56 w_lo.(...)
            pj2 = pjps.tile([W64, 512], F32, name=f"pj{h}", tag=f"pj{h}")
            for c in range(DC):
                lhsT = wt_s[:, c * W64:(c + 1) * W64]
                nc.tensor.matmul(pj2, lhsT, xth[c][:, tok],
                                 start=(c == 0), stop=False,
                                 skip_group_check=True)
                nc.tensor.matmul(pj2, lhsT, xtl[c][:, tok],
                                 start=False, stop=(c == DC - 1),
                                 skip_group_check=True)

            # combine hi+lo rows (DVE may read only one PSUM operand per op);
            # the explicit dep keeps this half's combine from being scheduled
            # ahead of the previous half's broadcasts on the in-order DVE
            # queue (head-of-line blocking while waiting on this half's psum)
            cmb = nc.vector.tensor_copy(plo[:, tok], pj2[W32:W32 + W24, :])
            if h > 0:
                add_dep_helper(cmb.ins, last_bcast.ins,
                               reason="DVE order: prev-half stores first")
            nc.vector.tensor_add(pjt[:, tok], pj2[0:W24, :], plo[:, tok])

            # back to token-major, fusing the bias add into the PSUM move
            for t in range(h * TH, (h + 1) * TH):
                bt = btps.tile([P, W24], F32, tag="bt")
                nc.tensor.transpose(bt, pjt[:, t * P:(t + 1) * P],
                                    ident_s[0:W24, 0:W24])
                nc.vector.tensor_add(proj[:, t * W24:(t + 1) * W24], bt,
                                     brep_s)

            # top-2 over the 8 gate logits (monotone in probs); per tile
            # since MAX8/FIND_INDEX8 reduce the whole free dim
            for t in range(h * TH, (h + 1) * TH):
                g = proj[:, t * W24:t * W24 + E]
                nc.vector.max(out=mx[:, t * E:(t + 1) * E], in_=g)
                nc.vector.max_index(out=mi[:, t * E:(t + 1) * E],
                                    in_max=mx[:, t * E:(t + 1) * E],
                                    in_values=g)

            # batched across this half's tiles
            nc.scalar.activation(p123[:, ts], mx3[:, ts, 0:2], ACTF.Sigmoid)
            nc.vector.tensor_tensor(
                msk3[:, 0, ts], proj3[:, ts, 0:E],
                mx3[:, ts, 0:1].to_broadcast([P, TH, E]), op=ALU.is_equal)
            nc.vector.tensor_tensor(
                msk3[:, 1, ts], proj3[:, ts, 0:E],
                mx3[:, ts, 1:2].to_broadcast([P, TH, E]), op=ALU.is_equal)
            nc.vector.tensor_mul(vv3[:, 0, ts], proj3[:, ts, E:2 * E],
                                 msk3[:, 0, ts])
            nc.vector.tensor_mul(vv3[:, 1, ts], proj3[:, ts, 2 * E:3 * E],
                                 msk3[:, 1, ts])
            nc.vector.reduce_sum(val3[:, :, ts], vv3[:, :, ts], axis=AX.X)

            # weighted = (val0*p0 + val1*p1) / (p0 + p1)
            nc.vector.tensor_mul(wv3[:, 0, ts], val3[:, 0, ts],
                                 p123[:, ts, 0])
            nc.vector.tensor_mul(wv3[:, 1, ts], val3[:, 1, ts],
                                 p123[:, ts, 1])
            nc.vector.tensor_add(num[:, ts], wv3[:, 0, ts], wv3[:, 1, ts])
            nc.vector.tensor_add(den[:, ts], p123[:, ts, 0], p123[:, ts, 1])
            nc.vector.reciprocal(rden[:, ts], den[:, ts])
            nc.vector.tensor_mul(wgt[:, ts], num[:, ts], rden[:, ts])

            # indices out: u32 -> i32 cast, one DMA per half
            nc.vector.tensor_copy(ii3[:, ts], mi3[:, ts, 0:2])
            nc.gpsimd.dma_start(idx3[:, ts], ii3[:, ts])

            # broadcast across the 1024 output features and store. DVE only:
            # ACT Copies would thrash the activation table, and GpSimd copies
            # run ~4us and stall DVE via port sharing. Stores ride the HWDGE
            # engines, whose queues are free again by now.
            for t in range(h * TH, (h + 1) * TH):
                obuf = opool.tile([P, D], F32, tag="obuf")
                src = wgt[:, t:t + 1].to_broadcast([P, D])
                last_bcast = nc.vector.tensor_copy(obuf, src)
                r0, rm, r1 = t * P, t * P + P // 2, (t + 1) * P
                nc.sync.dma_start(out[r0:rm, :], obuf[0:P // 2, :])
                nc.scalar.dma_start(out[rm:r1, :], obuf[P // 2:P, :])


def _prep_shared(gate_w, gate_b, expert_biases, expert_w, expert_b):
    """Host-side packing of the replicated small tensors."""
    w24 = np.concatenate(
        [gate_w, expert_w[:, 0, :], expert_w[:, 1, :]], axis=0)  # [24, 1024]
    w_hi = w24.astype(ml_dtypes.bfloat16)
    w_lo = (w24 - w_hi.astype(np.float32)).astype(ml_dtypes.bfloat16)
    pad = np.zeros((W32 - W24, D), ml_dtypes.bfloat16)
    w64 = np.concatenate([w_hi, pad, w_lo, pad], axis=0)         # [64, 1024]
    # wt[p, c*64 + j] = w64[j, c*128 + p]
    wt = np.ascontiguousarray(
        w64.T.reshape(DC, P, W64).transpose(1, 0, 2).reshape(P, DC * W64))
    b24 = np.concatenate(
        [gate_b + expert_biases, expert_b[:, 0], expert_b[:, 1]])  # [24]
    brep = np.broadcast_to(b24.astype(np.float32), (P, W24))
    ident = np.eye(P, dtype=np.float32)
    cst = np.ascontiguousarray(np.concatenate([brep, ident], axis=1))
    return wt, cst


def _build_module():
    nc = bacc.Bacc("TRN2", target_bir_lowering=False, debug=False,
                   num_devices=1)
    ins = {
        "xh": nc.dram_tensor("xh", [DC, P, TPC], BF16,
                             kind="ExternalInput").ap(),
        "xl": nc.dram_tensor("xl", [DC, P, TPC], BF16,
                             kind="ExternalInput").ap(),
        "wt": nc.dram_tensor("wt", [P, DC * W64], BF16,
                             kind="ExternalInput").ap(),
        "cst": nc.dram_tensor("cst", [P, W24 + P], F32,
                              kind="ExternalInput").ap(),
    }
    outs = {
        "out": nc.dram_tensor("out", [TPC, D], F32, kind="ExternalOutput").ap(),
        "idx": nc.dram_tensor("idx", [TPC, 2], I32, kind="ExternalOutput").ap(),
    }
    with TileContext(nc) as tc:
        build_kernel_body(nc, tc, ins, outs)
    nc.compile()
    return nc


_NC_CACHE = None


def _get_module():
    global _NC_CACHE
    if _NC_CACHE is None:
        _NC_CACHE = _build_module()
    return _NC_CACHE


def _run(inputs, trace=False, trace_kwargs=None):
    x = np.asarray(inputs["x"], np.float32)
    wt, cst = _prep_shared(
        np.asarray(inputs["gate_w"], np.float32),
        np.asarray(inputs["gate_b"], np.float32),
        np.asarray(inputs["expert_biases"], np.float32),
        np.asarray(inputs["expert_w"], np.float32),
        np.asarray(inputs["expert_b"], np.float32),
    )
    xf = x.reshape(TOK, D)
    x_hi = xf.astype(ml_dtypes.bfloat16)
    x_lo = (xf - x_hi.astype(np.float32)).astype(ml_dtypes.bfloat16)
    # pre-transposed chunk-major layout: xh[core][c][p][t] = x^T tiles
    x_hi = np.ascontiguousarray(
        x_hi.reshape(N_CORES, TPC, DC, P).transpose(0, 2, 3, 1))
    x_lo = np.ascontiguousarray(
        x_lo.reshape(N_CORES, TPC, DC, P).transpose(0, 2, 3, 1))
    shared = {"wt": wt, "cst": cst}
    in_maps = [
        {"xh": x_hi[c], "xl": x_lo[c], **shared}
        for c in range(N_CORES)
    ]
    nc = _get_module()
    kw = {}
    if trace:
        kw["trace"] = True
        kw["trace_cores"] = list(range(N_CORES))
        if trace_kwargs:
            kw["trace_kwargs"] = trace_kwargs
    res = bass_utils.run_bass_kernel_spmd(
        nc, in_maps, core_ids=list(range(N_CORES)), **kw)
    out = np.concatenate([res.results[c]["out"] for c in range(N_CORES)],
                         axis=0).reshape(B, S, D)
    idx = np.concatenate([res.results[c]["idx"] for c in range(N_CORES)],
                         axis=0).reshape(B, S, K)
    return (out.astype(np.float32), idx.astype(np.int32)), res


def kernel(**inputs):
    (out, idx), _ = _run(inputs)
    return out, idx
